# revision 1
# baseline (speedup 1.0000x reference)
"""Trainium2 Bass kernel for a post-LN transformer encoder layer.

Reference computation (fp32, per batch b):
    q,k,v = x@Wq+bq, x@Wk+bk, x@Wv+bv          (D=1024, H=16 heads, dk=64)
    attn  = softmax(q k^T / sqrt(dk)) v         (S=2048, mask is all-ones)
    h     = LN(x + attn@Wo + bo; g1, be1)
    out   = LN(h + relu(h@W1+b1)@W2 + b2; g2, be2)

Sharding: 8 cores, fully independent (no collectives). Core c owns batch
b=c//2, sequence half c%2 (1024 query tokens), and redundantly computes
K/V for its full batch (2048 keys) from a host-provided transposed copy
of x. The host rolls the token axis so each core's local tokens come
first (attention is permutation-invariant over keys, so K/V token order
doesn't matter as long as KT and V agree). Matmul operands are bf16
(fp32 accumulation in PSUM); softmax, layernorms and residuals are fp32.

Bias handling: bq/bk are applied as per-partition activation biases.
bv folds into bo (softmax rows sum to 1 => attn@(V+bv) = attn@V + bv),
and bo folds into the host-side residual copy of x. b1 is a per-partition
activation bias on the relu eviction. b2 is added on-device from a
partition-broadcast row.

Dataflow (all matmuls out = lhsT.T @ rhs, contraction on partitions):
  QT[D,tq]   = (Wq chunk).T @ xT_local    KT[D,tk] = (Wk chunk).T @ xT
  V[tk,D]    = (xT chunk).T @ Wv          (+ones column -> V_aug[...,65])
  scoresT[tk,tq] = KT_h.T @ QT_h ; expT = exp(0.125*scoresT)  (ACT, bf16)
  ctxT/denom: psum[65,tq] = sum_kc V_aug.T @ expT   (row 64 = softmax denom)
  CT = ctxT * bcast(1/denom)   (K=1 matmul broadcasts the recip row)
  attn_out[t,D] = CT.T @ Wo ; h = LN1(x' + attn_out) ; hT via PE transpose
  uT[dff,t] = relu((W1 chunk).T @ hT + b1) ; y[t,D] = uT.T @ W2 ; LN2(h+y)
"""

import numpy as np
import ml_dtypes

import concourse.bass as bass
import concourse.mybir as mybir
import concourse.tile as tile
from concourse.bass import ts
from concourse.bass_utils import run_bass_kernel_spmd
from concourse.masks import make_identity

BF16 = mybir.dt.bfloat16
F32 = mybir.dt.float32
F32R = mybir.dt.float32r
AF = mybir.ActivationFunctionType
ALU = mybir.AluOpType

D = 1024
DFF = 4096
H = 16
DK = 64
S_FULL = 2048
S_LOC = 1024
P = 128
NDC = D // P        # 8  feature chunks
NFC = DFF // P      # 32 ffn chunks
NKC = S_FULL // P   # 16 key chunks
NTC = S_LOC // P    # 8  local token chunks
NQT = S_LOC // 512  # 2 query tiles of 512
NKT = S_FULL // 512 # 4 key-token tiles of 512


# ---------------------------------------------------------------------------
# Multi-wait splitting: this walrus build rejects instructions carrying more
# than one sync-wait command. Tile occasionally emits several (notably the
# kernel-tail drain). Keep the last wait on the instruction and hoist the
# rest onto NoOps inserted just before it on the same engine queue.
_ctr = [0]


def _split_block(bb):
    out = []
    changed = False
    for inst in bb.instructions:
        si = inst.sync_info
        waits = list(si.on_wait) if si is not None and si.on_wait else []
        if len(waits) > 1:
            changed = True
            for w in waits[:-1]:
                _ctr[0] += 1
                nop = mybir.InstNoOp(name=f"waitfix-{_ctr[0]}", ins=[], outs=[])
                nop.engine = inst.engine
                nop.sync_info = mybir.SyncInfo(on_wait=[w], on_update=[])
                out.append(nop)
            inst.sync_info = mybir.SyncInfo(
                on_wait=[waits[-1]], on_update=list(si.on_update or [])
            )
        out.append(inst)
    if changed:
        bb.instructions = out
    return changed


def fix_multiwait(nc):
    for fn in nc.m.functions:
        for bb in fn.blocks:
            _split_block(bb)


# ---------------------------------------------------------------------------
def build_program(reps=1, waitfix=True, phases=('p1', 'attn', 'p3', 'ffn'),
                  attn_scores_only=False, attn_exp_on_dve=False,
                  attn_p0_zero=False, attn_psS_bufs=3, attn_merge_psB=True,
                  attn_pair_heads=True):
    nc = bass.Bass()

    xt_d = nc.dram_tensor("xt", [D, S_FULL], BF16, kind="ExternalInput")
    xloc_d = nc.dram_tensor("xloc", [S_LOC, D], F32, kind="ExternalInput")
    wq_d = nc.dram_tensor("wq", [D, D], BF16, kind="ExternalInput")
    wk_d = nc.dram_tensor("wk", [D, D], BF16, kind="ExternalInput")
    wv_d = nc.dram_tensor("wv", [D, D], BF16, kind="ExternalInput")
    wo_d = nc.dram_tensor("wo", [D, D], BF16, kind="ExternalInput")
    w1_d = nc.dram_tensor("w1", [D, DFF], BF16, kind="ExternalInput")
    w2_d = nc.dram_tensor("w2", [DFF, D], BF16, kind="ExternalInput")
    bqc_d = nc.dram_tensor("bqc", [P, NDC], F32, kind="ExternalInput")
    bkc_d = nc.dram_tensor("bkc", [P, NDC], F32, kind="ExternalInput")
    b1c_d = nc.dram_tensor("b1c", [P, NFC], F32, kind="ExternalInput")
    b2r_d = nc.dram_tensor("b2r", [1, D], F32, kind="ExternalInput")
    g1r_d = nc.dram_tensor("g1r", [1, D], F32, kind="ExternalInput")
    be1r_d = nc.dram_tensor("be1r", [1, D], F32, kind="ExternalInput")
    g2r_d = nc.dram_tensor("g2r", [1, D], F32, kind="ExternalInput")
    be2r_d = nc.dram_tensor("be2r", [1, D], F32, kind="ExternalInput")
    out_d = nc.dram_tensor("out", [S_LOC, D], F32, kind="ExternalOutput")

    xt_r = xt_d.rearrange("(dc p) t -> p dc t", p=P)
    wq_r = wq_d.rearrange("(dc p) o -> p dc o", p=P)
    wk_r = wk_d.rearrange("(dc p) o -> p dc o", p=P)
    wv_r = wv_d.rearrange("(dc p) o -> p dc o", p=P)
    wo_r = wo_d.rearrange("(dc p) o -> p dc o", p=P)
    w1_r = w1_d.rearrange("(dc p) f -> p dc f", p=P)
    w2_r = w2_d.rearrange("(fc p) o -> p fc o", p=P)

    def bcast_row(row_d):
        # [1, D] dram row -> partition-broadcast AP for DMA into [P, D]
        a = row_d[0:1, :]
        return bass.AP(tensor=a.tensor, offset=a.offset, ap=[[0, P], [1, D]])

    def layernorm_row(row, lnp, g_b, be_b, eps_t):
        st = lnp.tile([P, 2, 6], F32, tag="st")
        nc.vector.bn_stats(st[:, 0, :], row[:, 0:512])
        nc.vector.bn_stats(st[:, 1, :], row[:, 512:1024])
        mv = lnp.tile([P, 2], F32, tag="mv")
        nc.vector.bn_aggr(mv[:], st[:])
        nc.scalar.activation(mv[:, 1:2], mv[:, 1:2], AF.Sqrt, bias=eps_t[:])
        nc.vector.reciprocal(mv[:, 1:2], mv[:, 1:2])
        nc.vector.tensor_scalar(
            out=row,
            in0=row,
            scalar1=mv[:, 0:1],
            scalar2=mv[:, 1:2],
            op0=ALU.subtract,
            op1=ALU.mult,
        )
        nc.vector.tensor_mul(row, row, g_b[:])
        nc.vector.tensor_add(row, row, be_b[:])

    with tile.TileContext(nc) as tc:
        with (
            tc.tile_pool(name="top", bufs=1) as top,
            tc.tile_pool(name="lnp", bufs=4) as lnp,
        ):
            # ---- whole-kernel constants / persistents -------------------
            ident = top.tile([P, P], F32)
            make_identity(nc, ident)
            eps_t = top.tile([P, 1], F32)
            nc.vector.memset(eps_t, 1e-5)
            ones32 = top.tile([1, DK], F32)
            nc.vector.memset(ones32, 1.0)
            ones_r = top.tile([1, DK], F32R)
            with nc.allow_low_precision(reason="f32r round for PE broadcast"):
                nc.vector.tensor_copy(ones_r[:], ones32[:])
            bqc = top.tile([P, NDC], F32)
            nc.sync.dma_start(bqc[:], bqc_d[:])
            bkc = top.tile([P, NDC], F32)
            nc.sync.dma_start(bkc[:], bkc_d[:])
            b1c = top.tile([P, NFC], F32)
            nc.sync.dma_start(b1c[:], b1c_d[:])
            b2b = top.tile([P, D], F32)
            nc.sync.dma_start(b2b[:], bcast_row(b2r_d))
            g1b = top.tile([P, D], F32)
            nc.sync.dma_start(g1b[:], bcast_row(g1r_d))
            be1b = top.tile([P, D], F32)
            nc.sync.dma_start(be1b[:], bcast_row(be1r_d))
            g2b = top.tile([P, D], F32)
            nc.sync.dma_start(g2b[:], bcast_row(g2r_d))
            be2b = top.tile([P, D], F32)
            nc.sync.dma_start(be2b[:], bcast_row(be2r_d))

            hres = top.tile([P, NTC, D], F32)   # x' + attn_out, then LN1'd
            # hT split into qt halves so FFN1's first half can start
            # while LN1/transpose still run on the later token chunks
            hTa = top.tile([P, NDC, 512], BF16)
            hTb = top.tile([P, NDC, 512], BF16)

            for _rep in range(reps):
                with tc.tile_pool(name="poolA", bufs=1) as poolA:
                    QT = poolA.tile([P, NDC, S_LOC], BF16)
                    KT = poolA.tile([P, NDC, S_FULL], BF16)
                    VA = poolA.tile([P, NKC, H, DK + 1], BF16)
                    CT = poolA.tile([P, NDC, S_LOC], BF16)

                    # ---- phase 1: projections (xt streamed from DRAM) -------
                    if 'p1' in phases:
                      with (
                        tc.tile_pool(name="p1w", bufs=1) as p1w,
                        tc.tile_pool(name="p1x", bufs=6) as p1x,
                        tc.tile_pool(name="p1ps", bufs=8, space="PSUM") as p1ps,
                    ):
                        # QT (local tokens = first 1024 cols) and KT (all)
                        for w_r, bias_c, dst, ntiles in (
                            (wq_r, bqc, QT, NQT),
                            (wk_r, bkc, KT, NKT),
                        ):
                            w_sb = p1w.tile([P, NDC, D], BF16, tag="w")
                            nc.sync.dma_start(w_sb[:], w_r)
                            for kt in range(ntiles):
                                pss = [
                                    p1ps.tile([P, 512], F32, tag="pqk", name="pqk")
                                    for _ in range(NDC)
                                ]
                                for dc_in in range(NDC):
                                    xs = p1x.tile([P, 512], BF16, tag="xs")
                                    nc.sync.dma_start(
                                        xs[:], xt_r[:, dc_in, ts(kt, 512)]
                                    )
                                    for dc_out in range(NDC):
                                        nc.tensor.matmul(
                                            pss[dc_out][:],
                                            w_sb[:, dc_in, ts(dc_out, P)],
                                            xs[:],
                                            start=(dc_in == 0),
                                            stop=(dc_in == NDC - 1),
                                        )
                                for dc_out in range(NDC):
                                    nc.scalar.activation(
                                        dst[:, dc_out, ts(kt, 512)],
                                        pss[dc_out][:],
                                        AF.Identity,
                                        bias=bias_c[:, dc_out : dc_out + 1],
                                    )

                        # V (natural layout) + ones column
                        wv_sb = p1w.tile([P, NDC, D], BF16, tag="w")
                        nc.sync.dma_start(wv_sb[:], wv_r)
                        for kc in range(NKC):
                            xv = p1x.tile([P, NDC, P], BF16, tag="xv")
                            nc.sync.dma_start(xv[:], xt_r[:, :, ts(kc, P)])
                            for dt_ in range(2):
                                ps = p1ps.tile([P, 512], F32, tag="pqk", name="pv")
                                for dc in range(NDC):
                                    nc.tensor.matmul(
                                        ps[:],
                                        xv[:, dc, :],
                                        wv_sb[:, dc, ts(dt_, 512)],
                                        start=(dc == 0),
                                        stop=(dc == NDC - 1),
                                    )
                                nc.vector.tensor_copy(
                                    VA[:, kc, dt_ * 8 : (dt_ + 1) * 8, 0:DK],
                                    ps[:].rearrange("p (h d) -> p h d", h=8),
                                )
                        nc.vector.memset(VA[:, :, :, DK : DK + 1], 1.0)

                    # ---- phase 2: attention ---------------------------------
                    if 'attn' in phases:
                      with (
                        tc.tile_pool(name="expp", bufs=2) as expp,
                        tc.tile_pool(name="recp", bufs=2) as recp,
                        tc.tile_pool(name="psS", bufs=attn_psS_bufs, space="PSUM") as psS,
                        tc.tile_pool(name="psC", bufs=2, space="PSUM") as psC,
                        tc.tile_pool(name="psB", bufs=2, space="PSUM") as psB_,
                    ):
                        psB = psC if attn_merge_psB else psB_
                        # Scores for group g+1 are emitted before ctx of
                        # group g so the PE has matmul work while ACT is
                        # still exp-evicting the current group's scores.
                        # Score psums span 2 banks so each Exp handles 2
                        # key-chunks (fewer ACT instructions).
                        def emit_scores(h, qt):
                            p0 = 0 if attn_p0_zero else DK * (h % 2)
                            hc = h // 2
                            et = expp.tile([P, NKC, 512], BF16, tag="exp",
                                           name="exp")
                            for k2 in range(NKC // 2):
                                ps_s = psS.tile([P, 2, 512], F32, tag="ps_s",
                                                name="ps_s")
                                for j in range(2):
                                    kc = 2 * k2 + j
                                    nc.tensor.matmul(
                                        ps_s[:, j, :],
                                        KT[p0 : p0 + DK, hc, ts(kc, P)],
                                        QT[p0 : p0 + DK, hc, ts(qt, 512)],
                                        start=True,
                                        stop=True,
                                    )
                                if attn_exp_on_dve:
                                    nc.vector.tensor_copy(
                                        et[:, 2 * k2 : 2 * k2 + 2, :], ps_s[:]
                                    )
                                else:
                                    nc.scalar.activation(
                                        et[:, 2 * k2 : 2 * k2 + 2, :],
                                        ps_s[:],
                                        AF.Exp,
                                        scale=0.125,
                                    )
                            return et

                        def emit_ctx(h, qt, et):
                            if attn_scores_only:
                                return
                            p0 = 0 if attn_p0_zero else DK * (h % 2)
                            hc = h // 2
                            ps_c = psC.tile([P, 512], F32, tag="ps_c",
                                            name="ps_c")
                            for kc in range(NKC):
                                nc.tensor.matmul(
                                    ps_c[0 : DK + 1, :],
                                    VA[:, kc, h, :],
                                    et[:, kc, :],
                                    start=(kc == 0),
                                    stop=(kc == NKC - 1),
                                )
                            rec = recp.tile([1, 512], F32R, tag="rec",
                                            name="rec")
                            # f32r (~tf32) rounding of the softmax denoms:
                            # ~2e-4 relative, negligible vs bf16 exp tiles.
                            with nc.allow_low_precision(reason="f32r bcast"):
                                nc.vector.reciprocal(rec[:], ps_c[DK : DK + 1, :])
                            # K=1 matmul broadcasts the recip row over dk
                            # partitions (TensorTensor can't read two PSUMs,
                            # and SBUF APs can't partition-broadcast).
                            ps_b = psB.tile([DK, 512], F32, tag="ps_b" if not attn_merge_psB else "ps_c",
                                            name="ps_b")
                            nc.tensor.matmul(
                                ps_b[:], ones_r[:], rec[:], start=True, stop=True
                            )
                            recb = recp.tile([DK, 512], F32, tag="recb",
                                             name="recb")
                            nc.vector.tensor_copy(recb[:], ps_b[:])
                            nc.vector.tensor_mul(
                                CT[p0 : p0 + DK, hc, ts(qt, 512)],
                                ps_c[0:DK, :],
                                recb[:],
                            )

                        def run_unpaired():
                            groups = [(h, qt) for h in range(H) for qt in range(NQT)]
                            prev = None
                            for g in groups:
                                et_g = emit_scores(*g)
                                if prev is not None:
                                    emit_ctx(*prev[0], prev[1])
                                prev = (g, et_g)
                            emit_ctx(*prev[0], prev[1])

                        # Paired-head variant: the even/odd heads of chunk hc
                        # sit at base partitions 0/64, so their K=64 score
                        # matmuls map to different PE row-groups and run
                        # concurrently when emitted back-to-back (the psum
                        # halves are different banks). One Exp evicts both
                        # heads' tiles (1024 wide). Pipelined at half-key
                        # granularity so the exp pool stays at 2x16KB.
                        def emit_scores_half(hc, qt, half):
                            et = expp.tile([P, 8, 2, 512], BF16, tag="exp",
                                           name="exph")
                            for i8 in range(8):
                                kc = half * 8 + i8
                                ps_s = psS.tile([P, 2, 512], F32, tag="ps_s",
                                                name="ps_s")
                                for j in range(2):
                                    p0 = DK * j
                                    nc.tensor.matmul(
                                        ps_s[:, j, :],
                                        KT[p0 : p0 + DK, hc, ts(kc, P)],
                                        QT[p0 : p0 + DK, hc, ts(qt, 512)],
                                        start=True,
                                        stop=True,
                                    )
                                nc.scalar.activation(
                                    et[:, i8, :, :], ps_s[:], AF.Exp, scale=0.125
                                )
                            return et

                        cur_pc = {}

                        def emit_ctx_half(hc, qt, half, et):
                            if half == 0:
                                pcA = psC.tile([P, 512], F32, tag="ps_cA",
                                               name="ps_cA", bufs=1)
                                pcB = psC.tile([P, 512], F32, tag="ps_cB",
                                               name="ps_cB", bufs=1)
                                cur_pc[(hc, qt)] = (pcA, pcB)
                            pcA, pcB = cur_pc[(hc, qt)]
                            for j, pc in enumerate((pcA, pcB)):
                                h = 2 * hc + j
                                for i8 in range(8):
                                    kc = half * 8 + i8
                                    nc.tensor.matmul(
                                        pc[0 : DK + 1, :],
                                        VA[:, kc, h, :],
                                        et[:, i8, j, :],
                                        start=(kc == 0),
                                        stop=(kc == NKC - 1),
                                    )
                            if half == 1:
                                del cur_pc[(hc, qt)]
                                for j, pc in enumerate((pcA, pcB)):
                                    rec = recp.tile([1, 512], F32R, tag="rec",
                                                    name="rec")
                                    with nc.allow_low_precision(reason="f32r"):
                                        nc.vector.reciprocal(
                                            rec[:], pc[DK : DK + 1, :]
                                        )
                                    ps_b = psC.tile([DK, 512], F32, tag="ps_b2",
                                                    name="ps_b2", bufs=1)
                                    nc.tensor.matmul(
                                        ps_b[:], ones_r[:], rec[:],
                                        start=True, stop=True,
                                    )
                                    recb = recp.tile([DK, 512], F32, tag="recb",
                                                     name="recb")
                                    nc.vector.tensor_copy(recb[:], ps_b[:])
                                    nc.vector.tensor_mul(
                                        CT[DK * j : DK * j + DK, hc, ts(qt, 512)],
                                        pc[0:DK, :],
                                        recb[:],
                                    )

                        def ctx_ops(hc, qt, half):
                            # materialize psC accumulators + the ordered list
                            # of ctx matmuls for one half-unit
                            if half == 0:
                                pcA = psC.tile([P, 512], F32, tag="ps_cA",
                                               name="ps_cA", bufs=1)
                                pcB = psC.tile([P, 512], F32, tag="ps_cB",
                                               name="ps_cB", bufs=1)
                                cur_pc[(hc, qt)] = (pcA, pcB)
                            pcA, pcB = cur_pc[(hc, qt)]
                            ops = []
                            for j, pc in enumerate((pcA, pcB)):
                                h = 2 * hc + j
                                for i8 in range(8):
                                    ops.append((pc, h, half * 8 + i8, i8, j))
                            return ops

                        def emit_ctx_mm(op, et):
                            pc, h, kc, i8, j = op
                            nc.tensor.matmul(
                                pc[0 : DK + 1, :],
                                VA[:, kc, h, :],
                                et[:, i8, j, :],
                                start=(kc == 0),
                                stop=(kc == NKC - 1),
                            )

                        def emit_recip_tail(hc, qt):
                            pcA, pcB = cur_pc.pop((hc, qt))
                            for j, pc in enumerate((pcA, pcB)):
                                rec = recp.tile([1, 512], F32R, tag="rec",
                                                name="rec", bufs=1)
                                with nc.allow_low_precision(reason="f32r"):
                                    nc.vector.reciprocal(
                                        rec[:], pc[DK : DK + 1, :]
                                    )
                                # evict ctx to bf16 SBUF immediately so the
                                # psum accumulator frees after ~1.4us of DVE
                                # work; bufs=1 accumulators then suffice and
                                # the freed banks deepen the scores pipeline
                                ctr = recp.tile([DK, 512], BF16, tag="ctr",
                                                name="ctr")
                                nc.vector.tensor_copy(ctr[:], pc[0:DK, :])
                                # ps_b borrows a scores-pool slot
                                ps_b = psS.tile([DK, 512], F32, tag="ps_s",
                                                name="ps_b2")
                                nc.tensor.matmul(
                                    ps_b[:], ones_r[:], rec[:],
                                    start=True, stop=True,
                                )
                                recb = recp.tile([DK, 512], F32, tag="recb",
                                                 name="recb")
                                nc.vector.tensor_copy(recb[:], ps_b[:])
                                nc.vector.tensor_mul(
                                    CT[DK * j : DK * j + DK, hc, ts(qt, 512)],
                                    ctr[:],
                                    recb[:],
                                )

                        def run_paired():
                            # ctx matmuls of the previous half-unit are
                            # interleaved between this unit's score pairs so
                            # the PE fills its exp-wait gaps and ACT never
                            # idles between units.
                            units = [(hc, qt, half)
                                     for hc in range(H // 2)
                                     for qt in range(NQT)
                                     for half in (0, 1)]
                            prev = None
                            for u in units:
                                hc, qt, half = u
                                pops = ctx_ops(*prev[0]) if prev else []
                                pet = prev[1] if prev else None
                                pidx = 0
                                et = expp.tile([P, 8, 2, 512], BF16,
                                               tag="exp", name="exph")
                                for i8 in range(8):
                                    kc = half * 8 + i8
                                    ps_s = psS.tile([P, 2, 512], F32,
                                                    tag="ps_s", name="ps_s")
                                    for j in range(2):
                                        p0 = DK * j
                                        nc.tensor.matmul(
                                            ps_s[:, j, :],
                                            KT[p0 : p0 + DK, hc, ts(kc, P)],
                                            QT[p0 : p0 + DK, hc, ts(qt, 512)],
                                            start=True,
                                            stop=True,
                                        )
                                    nc.scalar.activation(
                                        et[:, i8, :, :], ps_s[:],
                                        AF.Exp, scale=0.125,
                                    )
                                    for _ in range(2):
                                        if pidx < len(pops):
                                            emit_ctx_mm(pops[pidx], pet)
                                            pidx += 1
                                while pidx < len(pops):
                                    emit_ctx_mm(pops[pidx], pet)
                                    pidx += 1
                                if prev is not None and prev[0][2] == 1:
                                    emit_recip_tail(prev[0][0], prev[0][1])
                                prev = (u, et)
                            for op in ctx_ops(*prev[0]):
                                emit_ctx_mm(op, prev[1])
                            emit_recip_tail(prev[0][0], prev[0][1])

                        if attn_pair_heads:
                            run_paired()
                        else:
                            run_unpaired()

                    # ---- phase 3: out-proj + residual + LN1 + hT ------------
                    if 'p3' in phases:
                      with (
                        tc.tile_pool(name="wop", bufs=1) as wop,
                        tc.tile_pool(name="xresp", bufs=3) as xresp,
                        tc.tile_pool(name="psO", bufs=4, space="PSUM") as psO,
                        tc.tile_pool(name="psT", bufs=4, space="PSUM") as psT,
                    ):
                        wo_sb = wop.tile([P, NDC, D], BF16)
                        nc.sync.dma_start(wo_sb[:], wo_r)
                        for tc_ in range(NTC):
                            for dt_ in range(2):
                                ps = psO.tile([P, 512], F32)
                                for dc in range(NDC):
                                    nc.tensor.matmul(
                                        ps[:],
                                        CT[:, dc, ts(tc_, P)],
                                        wo_sb[:, dc, ts(dt_, 512)],
                                        start=(dc == 0),
                                        stop=(dc == NDC - 1),
                                    )
                                xres = xresp.tile([P, 512], F32, tag="xres")
                                nc.sync.dma_start(
                                    xres[:], xloc_d[ts(tc_, P), ts(dt_, 512)]
                                )
                                nc.vector.tensor_add(
                                    hres[:, tc_, ts(dt_, 512)], ps[:], xres[:]
                                )
                            row = hres[:, tc_, :]
                            layernorm_row(row, lnp, g1b, be1b, eps_t)
                            hT_half = hTa if tc_ < 4 else hTb
                            tcol = tc_ % 4
                            for dc in range(NDC):
                                ps_t = psT.tile([P, P], F32)
                                nc.tensor.transpose(ps_t[:], row[:, ts(dc, P)], ident[:])
                                nc.vector.tensor_copy(
                                    hT_half[:, dc, ts(tcol, P)], ps_t[:]
                                )

                # ---- phase 4: FFN (poolA freed) -----------------------------
                if 'ffn' in phases:
                  with (
                    tc.tile_pool(name="uTp", bufs=1) as uTp,
                    tc.tile_pool(name="w1p", bufs=3) as w1p,
                    tc.tile_pool(name="w2p", bufs=1) as w2p,
                    tc.tile_pool(name="psF", bufs=4, space="PSUM") as psF,
                ):
                    uT = uTp.tile([P, NFC, S_LOC], BF16)
                    # prefetch the first w2 half under FFN1's compute
                    w2_sb0 = w2p.tile([P, NFC, 512], BF16, tag="w2", name="w2a")
                    nc.sync.dma_start(w2_sb0[:], w2_r[:, :, 0:512])
                    for fc in range(NFC):
                        w1_sb = w1p.tile([P, NDC, P], BF16, tag="w1")
                        nc.sync.dma_start(w1_sb[:], w1_r[:, :, ts(fc, P)])
                        for qt in range(NQT):
                            hT_half = hTa if qt == 0 else hTb
                            ps = psF.tile([P, 512], F32, tag="psf", name="psf")
                            for dc in range(NDC):
                                nc.tensor.matmul(
                                    ps[:],
                                    w1_sb[:, dc, :],
                                    hT_half[:, dc, :],
                                    start=(dc == 0),
                                    stop=(dc == NDC - 1),
                                )
                            nc.scalar.activation(
                                uT[:, fc, ts(qt, 512)],
                                ps[:],
                                AF.Relu,
                                bias=b1c[:, fc : fc + 1],
                            )

                    y = uTp.tile([P, NTC, D], F32)
                    with (
                        tc.tile_pool(name="psY", bufs=4, space="PSUM") as psY,
                    ):
                        for dt_ in range(2):
                            if dt_ == 0:
                                w2_sb = w2_sb0
                            else:
                                w2_sb = w2p.tile([P, NFC, 512], BF16, tag="w2",
                                                 name="w2b")
                                nc.sync.dma_start(
                                    w2_sb[:], w2_r[:, :, ts(dt_, 512)]
                                )
                            for tc_ in range(NTC):
                                ps = psY.tile([P, 512], F32)
                                for fc in range(NFC):
                                    nc.tensor.matmul(
                                        ps[:],
                                        uT[:, fc, ts(tc_, P)],
                                        w2_sb[:, fc, :],
                                        start=(fc == 0),
                                        stop=(fc == NFC - 1),
                                    )
                                nc.vector.tensor_add(
                                    y[:, tc_, ts(dt_, 512)],
                                    ps[:],
                                    hres[:, tc_, ts(dt_, 512)],
                                )
                                if dt_ == 1:
                                    # row complete -> LN2 on DVE overlaps
                                    # the remaining psY matmuls on PE
                                    row = y[:, tc_, :]
                                    nc.vector.tensor_add(row, row, b2b[:])
                                    layernorm_row(row, lnp, g2b, be2b, eps_t)
                                    nc.sync.dma_start(
                                        out_d[ts(tc_, P), :], row
                                    )

    if waitfix:
        fix_multiwait(nc)
    return nc


_NC = None
LAST_RESULTS = None  # BassKernelResults of the most recent kernel() call


def kernel(x, mask, Wq, bq, Wk, bk, Wv, bv, Wo, bo, W1, b1, W2, b2, g1, be1, g2, be2):
    global _NC
    if _NC is None:
        _NC = build_program()
    nc = _NC

    bf = ml_dtypes.bfloat16
    x = np.asarray(x, np.float32)
    Wo32 = np.asarray(Wo, np.float32)
    bo_eff = np.asarray(bo, np.float32) + np.asarray(bv, np.float32) @ Wo32

    def col(b_, n):  # [n*128] -> [128, n] column layout
        return np.ascontiguousarray(np.asarray(b_, np.float32).reshape(n, P).T)

    def row(b_):
        return np.ascontiguousarray(np.asarray(b_, np.float32).reshape(1, -1))

    shared = {
        "wq": np.ascontiguousarray(np.asarray(Wq, np.float32).astype(bf)),
        "wk": np.ascontiguousarray(np.asarray(Wk, np.float32).astype(bf)),
        "wv": np.ascontiguousarray(np.asarray(Wv, np.float32).astype(bf)),
        "wo": np.ascontiguousarray(Wo32.astype(bf)),
        "w1": np.ascontiguousarray(np.asarray(W1, np.float32).astype(bf)),
        "w2": np.ascontiguousarray(np.asarray(W2, np.float32).astype(bf)),
        "bqc": col(bq, NDC),
        "bkc": col(bk, NDC),
        "b1c": col(b1, NFC),
        "b2r": row(b2),
        "g1r": row(g1),
        "be1r": row(be1),
        "g2r": row(g2),
        "be2r": row(be2),
    }

    in_maps = []
    for c in range(8):
        b_, hf = c // 2, c % 2
        xb = x[b_]  # [2048, 1024]
        loc = xb[hf * S_LOC : (hf + 1) * S_LOC, :]
        rem = xb[(1 - hf) * S_LOC : (2 - hf) * S_LOC, :]
        m = dict(shared)
        # token axis rolled: local tokens first (keys are permutation-inv.)
        m["xt"] = np.ascontiguousarray(
            np.concatenate([loc, rem], axis=0).T.astype(bf)
        )
        m["xloc"] = np.ascontiguousarray(loc + bo_eff[None, :])
        in_maps.append(m)

    res = run_bass_kernel_spmd(nc, in_maps, list(range(8)))
    global LAST_RESULTS
    LAST_RESULTS = res

    out = np.empty((4, S_FULL, D), np.float32)
    for c in range(8):
        b_, hf = c // 2, c % 2
        out[b_, hf * S_LOC : (hf + 1) * S_LOC, :] = res.results[c]["out"]
    return out



# revision 22
# speedup vs baseline: 1.6606x; 1.6606x over previous
"""Trainium2 Bass kernel for a post-LN transformer encoder layer.

Reference computation (fp32, per batch b):
    q,k,v = x@Wq+bq, x@Wk+bk, x@Wv+bv          (D=1024, H=16 heads, dk=64)
    attn  = softmax(q k^T / sqrt(dk)) v         (S=2048, mask is all-ones)
    h     = LN(x + attn@Wo + bo; g1, be1)
    out   = LN(h + relu(h@W1+b1)@W2 + b2; g2, be2)

Sharding: 8 cores, fully independent (no collectives). Core c owns batch
b=c//2, sequence half c%2 (1024 query tokens), and redundantly computes
K/V for its full batch (2048 keys) from a host-provided transposed fp8
copy of x (token axis rolled so local tokens come first; attention is
permutation-invariant over keys).

Precision scheme: fp8 e4m3 everywhere the error doesn't reach the output
(the whole attention path contributes ~2% of the residual magnitude) with
DoubleRow double-pumped matmuls (two stacked K=128 k-tiles per
instruction, 2x PE throughput). FFN1 stays bf16 (its input quantization
error would land directly on the output); FFN2 is fp8 DoubleRow. All
quantization scales are powers of two folded into host-side weight prep,
activation-eviction scale immediates, and the LN epsilon, so no extra
on-device ops are spent on scaling. Softmax exp is split across three
engines: exact Exp on ACT, and a Schraudolph bit-trick exp
(bits = scale*score + offset, f32->int8 saturating convert, bitcast to
fp8e4) on Pool and DVE; exp errors cancel between the ctx numerator and
the ones-column denominator.

Scale ledger (powers of 2; value stored = true value * 2^s):
  xt:5  wq/wk/wv:12  QT/KT/VA:5  exp:0  CT:10  wo:3  xloc/hres/hT:13
  w1:0(bf16)  uT:6  w2:7  FFN2 psum:13  LN eps: +26  output: unscaled
"""

import numpy as np
import ml_dtypes

import concourse.bass as bass
import concourse.mybir as mybir
import concourse.tile as tile
from concourse.bass import ts
from concourse.bass_utils import run_bass_kernel_spmd
from concourse.masks import make_identity

BF16 = mybir.dt.bfloat16
F32 = mybir.dt.float32
F32R = mybir.dt.float32r
F8 = mybir.dt.float8e4
I8 = mybir.dt.int8
AF = mybir.ActivationFunctionType
ALU = mybir.AluOpType
DR = mybir.MatmulPerfMode.DoubleRow

D = 1024
DFF = 4096
H = 16
DK = 64
S_FULL = 2048
S_LOC = 1024
P = 128
NDC = D // P        # 8  feature chunks
NFC = DFF // P      # 32 ffn chunks
NKC = S_FULL // P   # 16 key chunks
NTC = S_LOC // P    # 8  local token chunks
NQT = S_LOC // 512  # 2 query tiles of 512

# ---- scale ledger ---------------------------------------------------------
SX = 5        # xt fp8
SW = 12       # wq/wk/wv fp8
SQKV = 5      # QT/KT/VA fp8
SCT = 10      # CT fp8
SWO = 3       # wo fp8
SH = 13       # xloc/hres/hT
SU = 6        # uT fp8
SW2 = 7       # w2 fp8
SC_QKV_EVICT = 2.0 ** (SQKV - SX - SW)      # psum -> fp8
SC_EXP = 0.125 * 2.0 ** (-2 * SQKV)         # ACT exp scale on scores psum
SC_RELU = 2.0 ** (SU - SH)                  # FFN1 psum -> uT fp8
EPS_LN = 1e-5 * 2.0 ** (2 * SH)             # rows carry 2^SH
LOG2E = 1.4426950408889634
A8 = 8.0 * LOG2E * 0.125 * 2.0 ** (-2 * SQKV)   # Schraudolph mult
B8 = 8.0 * 7.0 - 0.5                            # Schraudolph offset
ONES_R_VAL = float(2 ** (SCT - SQKV))       # PE-bcast recip row scale

# exp engine per key chunk: A=ACT exact, D=DVE Schraudolph. GPSIMD (Pool)
# cannot read PSUM on TRN2, so it gets only SBUF-side work (LN affine etc).
EXP_PAT = ['A', 'D', 'A', 'D', 'A', 'A', 'D', 'A',
           'D', 'A', 'A', 'D', 'A', 'D', 'A', 'D']


# ---------------------------------------------------------------------------
# Multi-wait splitting: this walrus build rejects instructions carrying more
# than one sync-wait command. Tile occasionally emits several (notably the
# kernel-tail drain). Keep the last wait on the instruction and hoist the
# rest onto NoOps inserted just before it on the same engine queue.
_ctr = [0]


def _split_block(bb):
    out = []
    changed = False
    for inst in bb.instructions:
        si = inst.sync_info
        waits = list(si.on_wait) if si is not None and si.on_wait else []
        if len(waits) > 1:
            changed = True
            for w in waits[:-1]:
                _ctr[0] += 1
                nop = mybir.InstNoOp(name=f"waitfix-{_ctr[0]}", ins=[], outs=[])
                nop.engine = inst.engine
                nop.sync_info = mybir.SyncInfo(on_wait=[w], on_update=[])
                out.append(nop)
            inst.sync_info = mybir.SyncInfo(
                on_wait=[waits[-1]], on_update=list(si.on_update or [])
            )
        out.append(inst)
    if changed:
        bb.instructions = out
    return changed


def fix_multiwait(nc):
    for fn in nc.m.functions:
        for bb in fn.blocks:
            _split_block(bb)


# ---------------------------------------------------------------------------
def build_program(reps=1, waitfix=True, phases=('p1', 'attn', 'p3', 'ffn')):
    nc = bass.Bass()

    xt_d = nc.dram_tensor("xt", [D, S_FULL], F8, kind="ExternalInput")
    xloc_d = nc.dram_tensor("xloc", [S_LOC, D], F32, kind="ExternalInput")
    wq_d = nc.dram_tensor("wq", [D, D], F8, kind="ExternalInput")
    wk_d = nc.dram_tensor("wk", [D, D], F8, kind="ExternalInput")
    wv_d = nc.dram_tensor("wv", [D, D], F8, kind="ExternalInput")
    wo_d = nc.dram_tensor("wo", [D, D], F8, kind="ExternalInput")
    w1_d = nc.dram_tensor("w1", [D, DFF], BF16, kind="ExternalInput")
    w2_d = nc.dram_tensor("w2", [DFF, D], F8, kind="ExternalInput")
    bqc_d = nc.dram_tensor("bqc", [P, NDC], F32, kind="ExternalInput")
    bkc_d = nc.dram_tensor("bkc", [P, NDC], F32, kind="ExternalInput")
    b1c_d = nc.dram_tensor("b1c", [P, NFC], F32, kind="ExternalInput")
    b2r_d = nc.dram_tensor("b2r", [1, D], F32, kind="ExternalInput")
    g1r_d = nc.dram_tensor("g1r", [1, D], F32, kind="ExternalInput")
    be1r_d = nc.dram_tensor("be1r", [1, D], F32, kind="ExternalInput")
    g2r_d = nc.dram_tensor("g2r", [1, D], F32, kind="ExternalInput")
    be2r_d = nc.dram_tensor("be2r", [1, D], F32, kind="ExternalInput")
    out_d = nc.dram_tensor("out", [S_LOC, D], F32, kind="ExternalOutput")

    xt_r = xt_d.rearrange("(dc p) t -> p dc t", p=P)
    wq_r = wq_d.rearrange("(dc p) o -> p dc o", p=P)
    wk_r = wk_d.rearrange("(dc p) o -> p dc o", p=P)
    wv_r = wv_d.rearrange("(dc p) o -> p dc o", p=P)
    wo_r = wo_d.rearrange("(dc p) o -> p dc o", p=P)
    w1_r = w1_d.rearrange("(dc p) f -> p dc f", p=P)
    w2_r = w2_d.rearrange("(fc p) o -> p fc o", p=P)

    def bcast_row(row_d):
        # [1, D] dram row -> partition-broadcast AP for DMA into [P, D]
        a = row_d[0:1, :]
        return bass.AP(tensor=a.tensor, offset=a.offset, ap=[[0, P], [1, D]])

    with tile.TileContext(nc) as tc:
        with (
            tc.tile_pool(name="top", bufs=1) as top,
            tc.tile_pool(name="lnp", bufs=4) as lnp,
        ):
            # ---- whole-kernel constants / persistents -------------------
            ident = top.tile([P, P], F32)
            make_identity(nc, ident)
            eps_t = top.tile([P, 1], F32)
            nc.vector.memset(eps_t, EPS_LN)
            ones32 = top.tile([1, DK], F32)
            nc.vector.memset(ones32, ONES_R_VAL)
            ones_r = top.tile([1, DK], F32R)
            with nc.allow_low_precision(reason="f32r round for PE broadcast"):
                nc.vector.tensor_copy(ones_r[:], ones32[:])
            bqc = top.tile([P, NDC], F32)
            nc.sync.dma_start(bqc[:], bqc_d[:])
            bkc = top.tile([P, NDC], F32)
            nc.sync.dma_start(bkc[:], bkc_d[:])
            # the remaining constant rows (2.5MB) are DMA'd after p1's
            # critical wq/xt transfers so they don't delay the first matmul
            b1c = top.tile([P, NFC], F32)
            b2b = top.tile([P, D], F32)
            g1b = top.tile([P, D], F32)
            be1b = top.tile([P, D], F32)
            g2b = top.tile([P, D], F32)
            be2b = top.tile([P, D], F32)

            def persist_dmas():
                nc.sync.dma_start(b1c[:], b1c_d[:])
                nc.sync.dma_start(b2b[:], bcast_row(b2r_d))
                nc.sync.dma_start(g1b[:], bcast_row(g1r_d))
                nc.sync.dma_start(be1b[:], bcast_row(be1r_d))
                nc.sync.dma_start(g2b[:], bcast_row(g2r_d))
                nc.sync.dma_start(be2b[:], bcast_row(be2r_d))

            hres = top.tile([P, NTC, D], F32)   # 2^SH * (x+attn), then LN1'd
            # hT split into qt halves so FFN1's first half can start
            # while LN1/transpose still run on the later token chunks
            hTa = top.tile([P, NDC, 512], BF16)
            hTb = top.tile([P, NDC, 512], BF16)

            def layernorm_row(row, g_b, be_b, stand=None, mul=None, add=None):
                # stats+recip on DVE; sqrt on ACT; standardize+affine on
                # the given engines (default Pool, the SBUF-only engine)
                stand = stand or nc.gpsimd
                mul = mul or nc.gpsimd
                add = add or nc.gpsimd
                st = lnp.tile([P, 2, 6], F32, tag="st")
                nc.vector.bn_stats(st[:, 0, :], row[:, 0:512])
                nc.vector.bn_stats(st[:, 1, :], row[:, 512:1024])
                mv = lnp.tile([P, 2], F32, tag="mv")
                nc.vector.bn_aggr(mv[:], st[:])
                nc.scalar.activation(mv[:, 1:2], mv[:, 1:2], AF.Sqrt,
                                     bias=eps_t[:])
                nc.vector.reciprocal(mv[:, 1:2], mv[:, 1:2])
                negmur = lnp.tile([P, 1], F32, tag="negmur")
                nc.vector.tensor_scalar(
                    out=negmur[:], in0=mv[:, 0:1],
                    scalar1=mv[:, 1:2], scalar2=-1.0,
                    op0=ALU.mult, op1=ALU.mult,
                )
                # row = row * (1/sigma) + (-mu/sigma)
                if stand is nc.scalar:
                    nc.scalar.activation(row, row, AF.Identity,
                                         bias=negmur[:], scale=mv[:, 1:2])
                else:
                    stand.tensor_scalar(
                        out=row, in0=row,
                        scalar1=mv[:, 1:2], scalar2=negmur[:],
                        op0=ALU.mult, op1=ALU.add,
                    )
                mul.tensor_mul(row, row, g_b[:])
                add.tensor_add(row, row, be_b[:])

            for _rep in range(reps):
                with tc.tile_pool(name="poolA", bufs=1) as poolA:
                    QT = poolA.tile([P, NDC, S_LOC], F8)
                    KT = poolA.tile([P, NDC, S_FULL], F8)
                    VA = poolA.tile([P, NKC, H, DK + 1], F8)
                    CT = poolA.tile([P, NDC, S_LOC], F8)

                    # ---- phase 1: projections -------------------------------
                    if 'p1' in phases:
                      with (
                        tc.tile_pool(name="p1w", bufs=1) as p1w,
                        tc.tile_pool(name="p1x", bufs=1) as p1x,
                        tc.tile_pool(name="p1ps", bufs=8, space="PSUM") as p1ps,
                      ):
                        # one fp8 copy of x^T in SBUF, quartered by tokens
                        xt_q = [p1x.tile([P, NDC, 512], F8, name=f"xtq{i}")
                                for i in range(4)]
                        wq_sb = p1w.tile([P, NDC, D], F8, name="wq_sb")
                        nc.sync.dma_start(xt_q[0][:], xt_r[:, :, 0:512])
                        nc.sync.dma_start(wq_sb[:], wq_r)
                        for i in range(1, 4):
                            nc.sync.dma_start(xt_q[i][:],
                                              xt_r[:, :, ts(i, 512)])
                        wk_sb = p1w.tile([P, NDC, D], F8, name="wk_sb")
                        nc.sync.dma_start(wk_sb[:], wk_r)
                        wv_sb = p1w.tile([P, NDC, D], F8, name="wv_sb")
                        nc.sync.dma_start(wv_sb[:], wv_r)
                        if _rep == 0:
                            persist_dmas()

                        # QT (local tokens = first 1024 cols) and KT (all)
                        for w_sb, bias_c, dst, ntiles, eng in (
                            (wq_sb, bqc, QT, NQT, 'D'),
                            (wk_sb, bkc, KT, NKC // 4, 'A'),
                        ):
                            for kt in range(ntiles):
                                pss = [
                                    p1ps.tile([P, 512], F32, tag="pqk",
                                              name="pqk")
                                    for _ in range(NDC)
                                ]
                                for dcp in range(NDC // 2):
                                    for dc_out in range(NDC):
                                        nc.tensor.matmul(
                                            pss[dc_out][:],
                                            w_sb[:, 2 * dcp : 2 * dcp + 2,
                                                 ts(dc_out, P)],
                                            xt_q[kt][:, 2 * dcp : 2 * dcp + 2, :],
                                            start=(dcp == 0),
                                            stop=(dcp == NDC // 2 - 1),
                                            perf_mode=DR,
                                        )
                                for dc_out in range(NDC):
                                    if eng == 'A':
                                        nc.scalar.activation(
                                            dst[:, dc_out, ts(kt, 512)],
                                            pss[dc_out][:],
                                            AF.Identity,
                                            bias=bias_c[:, dc_out : dc_out + 1],
                                            scale=SC_QKV_EVICT,
                                        )
                                    else:
                                        nc.vector.tensor_scalar(
                                            out=dst[:, dc_out, ts(kt, 512)],
                                            in0=pss[dc_out][:],
                                            scalar1=SC_QKV_EVICT,
                                            scalar2=bias_c[:, dc_out : dc_out + 1],
                                            op0=ALU.mult,
                                            op1=ALU.add,
                                        )

                        # V (natural layout) + ones column
                        for kc in range(NKC):
                            xq = xt_q[kc // 4]
                            for dt_ in range(2):
                                ps = p1ps.tile([P, 512], F32, tag="pqk",
                                               name="pv")
                                for dcp in range(NDC // 2):
                                    nc.tensor.matmul(
                                        ps[:],
                                        xq[:, 2 * dcp : 2 * dcp + 2,
                                           ts(kc % 4, P)],
                                        wv_sb[:, 2 * dcp : 2 * dcp + 2,
                                              ts(dt_, 512)],
                                        start=(dcp == 0),
                                        stop=(dcp == NDC // 2 - 1),
                                        perf_mode=DR,
                                    )
                                nc.scalar.activation(
                                    VA[:, kc, dt_ * 8 : (dt_ + 1) * 8, 0:DK],
                                    ps[:].rearrange("p (h d) -> p h d", h=8),
                                    AF.Copy,
                                    scale=SC_QKV_EVICT,
                                )
                        nc.gpsimd.memset(VA[:, :, :, DK : DK + 1], 1.0)

                    # prefetch wo under the attention phase
                    wop = poolA
                    wo_sb = wop.tile([P, NDC, D], F8)
                    if 'p3' in phases:
                        nc.sync.dma_start(wo_sb[:], wo_r)

                    # ---- phase 2: attention ---------------------------------
                    if 'attn' in phases:
                      with (
                        tc.tile_pool(name="expp", bufs=2) as expp,
                        tc.tile_pool(name="recp", bufs=2) as recp,
                        tc.tile_pool(name="psS", bufs=3, space="PSUM") as psS,
                        tc.tile_pool(name="psC", bufs=1, space="PSUM") as psC,
                      ):
                        cur_pc = {}

                        def ctx_ops(hc, qt, et):
                            pcA = psC.tile([P, 512], F32, tag="ps_cA",
                                           name="ps_cA", bufs=1)
                            pcB = psC.tile([P, 512], F32, tag="ps_cB",
                                           name="ps_cB", bufs=1)
                            cur_pc[(hc, qt)] = (pcA, pcB)
                            ops = []
                            for j, pc in enumerate((pcA, pcB)):
                                h = 2 * hc + j
                                for kp in range(NKC // 2):
                                    ops.append((pc, h, kp, j, et))
                            return ops

                        def emit_ctx_mm(op):
                            pc, h, kp, j, et = op
                            nc.tensor.matmul(
                                pc[0 : DK + 1, :],
                                VA[:, 2 * kp : 2 * kp + 2, h, :],
                                et[:, 2 * kp : 2 * kp + 2, j, :],
                                start=(kp == 0),
                                stop=(kp == NKC // 2 - 1),
                                perf_mode=DR,
                            )

                        def emit_tail(hc, qt):
                            pcA, pcB = cur_pc.pop((hc, qt))
                            for j, pc in enumerate((pcA, pcB)):
                                rec = recp.tile([1, 512], F32R, tag="rec",
                                                name="rec")
                                with nc.allow_low_precision(reason="f32r"):
                                    nc.vector.reciprocal(
                                        rec[:], pc[DK : DK + 1, :]
                                    )
                                # K=1 matmul broadcasts the recip row over dk
                                # partitions (TensorTensor can't read two
                                # PSUMs, SBUF APs can't partition-broadcast)
                                ps_b = psS.tile([DK, 2, 512], F32, tag="ps_s",
                                                name="ps_b")
                                nc.tensor.matmul(
                                    ps_b[:, 0, :], ones_r[:], rec[:],
                                    start=True, stop=True,
                                )
                                recb = recp.tile([DK, 512], F32, tag="recb",
                                                 name="recb")
                                nc.scalar.activation(recb[:], ps_b[:, 0, :],
                                                     AF.Copy)
                                nc.vector.tensor_mul(
                                    CT[DK * j : DK * j + DK, hc, ts(qt, 512)],
                                    pc[0:DK, :],
                                    recb[:],
                                )

                        units = [(hc, qt) for qt in range(NQT)
                                 for hc in range(H // 2)]
                        prev = None
                        for u in units:
                            hc, qt = u
                            et = expp.tile([P, NKC, 2, 512], F8, tag="exp",
                                           name="exp")
                            pops = ctx_ops(*prev) if prev else []
                            pidx = 0
                            for kc in range(NKC):
                                ps_s = psS.tile([P, 2, 512], F32, tag="ps_s",
                                                name="ps_s")
                                for j in range(2):
                                    p0 = DK * j
                                    nc.tensor.matmul(
                                        ps_s[:, j, :],
                                        KT[p0 : p0 + DK, hc, ts(kc, P)],
                                        QT[p0 : p0 + DK, hc, ts(qt, 512)],
                                        start=True,
                                        stop=True,
                                    )
                                eng = EXP_PAT[kc]
                                dst = et[:, kc, :, :]
                                if eng == 'A':
                                    nc.scalar.activation(
                                        dst, ps_s[:], AF.Exp, scale=SC_EXP
                                    )
                                else:
                                    nc.vector.tensor_scalar(
                                        out=dst.bitcast(I8),
                                        in0=ps_s[:],
                                        scalar1=A8,
                                        scalar2=B8,
                                        op0=ALU.mult,
                                        op1=ALU.add,
                                    )
                                if pidx < len(pops):
                                    emit_ctx_mm(pops[pidx])
                                    pidx += 1
                            while pidx < len(pops):
                                emit_ctx_mm(pops[pidx])
                                pidx += 1
                            if prev is not None:
                                emit_tail(*prev[:2])
                            prev = (hc, qt, et)
                        for op in ctx_ops(*prev):
                            emit_ctx_mm(op)
                        emit_tail(*prev[:2])

                    # ---- phase 3: out-proj + residual + LN1 + hT ------------
                    # rows 0-3: full (LN1 + transpose). rows 4-7: out-proj +
                    # residual only; their LN1/transpose is woven into FFN1
                    # where DVE/Pool are otherwise idle.
                    def finish_row(tc_, psT, stand=None, mul=None, add=None):
                        row = hres[:, tc_, :]
                        layernorm_row(row, g1b, be1b, stand, mul, add)
                        hT_half = hTa if tc_ < 4 else hTb
                        tcol = tc_ % 4
                        for dcq in range(2):
                            ps_t = psT.tile([P, 4, P], F32, tag="ps_t",
                                            name="ps_t")
                            for di in range(4):
                                nc.tensor.transpose(
                                    ps_t[:, di, :],
                                    row[:, ts(4 * dcq + di, P)],
                                    ident[:],
                                )
                            if dcq == 0:
                                nc.vector.tensor_copy(
                                    hT_half[:, 0:4, ts(tcol, P)], ps_t[:]
                                )
                            else:
                                nc.scalar.activation(
                                    hT_half[:, 4:8, ts(tcol, P)], ps_t[:],
                                    AF.Copy,
                                )
                        # pre-add b2 to the residual now that hT holds the
                        # true h'; keeps it off the FFN2 tail path
                        nc.gpsimd.tensor_add(row, row, b2b[:])

                    if 'p3' in phases:
                      with (
                        tc.tile_pool(name="xresp", bufs=3) as xresp,
                        tc.tile_pool(name="psO", bufs=4, space="PSUM") as psO,
                        tc.tile_pool(name="psT", bufs=2, space="PSUM") as psT,
                      ):
                        for tc_ in range(NTC):
                            for dt_ in range(2):
                                ps = psO.tile([P, 512], F32)
                                for dcp in range(NDC // 2):
                                    nc.tensor.matmul(
                                        ps[:],
                                        CT[:, 2 * dcp : 2 * dcp + 2,
                                           ts(tc_, P)],
                                        wo_sb[:, 2 * dcp : 2 * dcp + 2,
                                              ts(dt_, 512)],
                                        start=(dcp == 0),
                                        stop=(dcp == NDC // 2 - 1),
                                        perf_mode=DR,
                                    )
                                xres = xresp.tile([P, 512], F32, tag="xres")
                                nc.sync.dma_start(
                                    xres[:], xloc_d[ts(tc_, P), ts(dt_, 512)]
                                )
                                nc.vector.tensor_add(
                                    hres[:, tc_, ts(dt_, 512)], ps[:], xres[:]
                                )
                            if tc_ < 4:
                                finish_row(tc_, psT)

                # ---- phase 4: FFN (poolA freed) -----------------------------
                if 'ffn' in phases:
                  with (
                    tc.tile_pool(name="uTp", bufs=1) as uTp,
                    tc.tile_pool(name="w1p", bufs=3) as w1p,
                    tc.tile_pool(name="w2p", bufs=1) as w2p,
                    tc.tile_pool(name="psF", bufs=4, space="PSUM") as psF,
                    tc.tile_pool(name="psT2", bufs=2, space="PSUM") as psT2,
                  ):
                    uT = uTp.tile([P, NFC, S_LOC], F8)
                    # w2 is 4MB fp8: fetch both halves under FFN1's compute
                    w2_sb = w2p.tile([P, NFC, D], F8, name="w2sb")
                    nc.sync.dma_start(w2_sb[:], w2_r)

                    def ffn1_mm(fc, qt, w1_sb):
                        hT_half = hTa if qt == 0 else hTb
                        ps = psF.tile([P, 512], F32, tag="psf", name="psf")
                        for dc in range(NDC):
                            nc.tensor.matmul(
                                ps[:],
                                w1_sb[:, dc, :],
                                hT_half[:, dc, :],
                                start=(dc == 0),
                                stop=(dc == NDC - 1),
                            )
                        nc.scalar.activation(
                            uT[:, fc, ts(qt, 512)],
                            ps[:],
                            AF.Relu,
                            bias=b1c[:, fc : fc + 1],
                            scale=SC_RELU,
                        )

                    # pass A: fc 0-7 on the ready hTa half, weaving in the
                    # deferred LN1/transpose of rows 4-7
                    for fc in range(8):
                        w1_sb = w1p.tile([P, NDC, P], BF16, tag="w1")
                        nc.sync.dma_start(w1_sb[:], w1_r[:, :, ts(fc, P)])
                        ffn1_mm(fc, 0, w1_sb)
                        if fc % 2 == 1 and 'p3' in phases:
                            finish_row(4 + fc // 2, psT2,
                                       stand=nc.scalar, mul=nc.vector,
                                       add=nc.gpsimd)
                    # pass B: fc 8-31, both token halves
                    for fc in range(8, NFC):
                        w1_sb = w1p.tile([P, NDC, P], BF16, tag="w1")
                        nc.sync.dma_start(w1_sb[:], w1_r[:, :, ts(fc, P)])
                        ffn1_mm(fc, 0, w1_sb)
                        ffn1_mm(fc, 1, w1_sb)
                    # pass C: fc 0-7 on hTb (w1 restreamed, +2MB DMA)
                    for fc in range(8):
                        w1_sb = w1p.tile([P, NDC, P], BF16, tag="w1")
                        nc.sync.dma_start(w1_sb[:], w1_r[:, :, ts(fc, P)])
                        ffn1_mm(fc, 1, w1_sb)

                    y = uTp.tile([P, NTC, D], F32)
                    with (
                        tc.tile_pool(name="psY", bufs=4, space="PSUM") as psY,
                    ):
                        # token-chunk outer so each row's LN2 + store
                        # pipelines under the next rows' matmuls
                        for tc_ in range(NTC):
                            for dt_ in range(2):
                                ps = psY.tile([P, 512], F32)
                                for fcp in range(NFC // 2):
                                    nc.tensor.matmul(
                                        ps[:],
                                        uT[:, 2 * fcp : 2 * fcp + 2,
                                           ts(tc_, P)],
                                        w2_sb[:, 2 * fcp : 2 * fcp + 2,
                                              ts(dt_, 512)],
                                        start=(fcp == 0),
                                        stop=(fcp == NFC // 2 - 1),
                                        perf_mode=DR,
                                    )
                                nc.vector.tensor_add(
                                    y[:, tc_, ts(dt_, 512)],
                                    ps[:],
                                    hres[:, tc_, ts(dt_, 512)],
                                )
                            row = y[:, tc_, :]
                            layernorm_row(row, g2b, be2b)
                            nc.sync.dma_start(out_d[ts(tc_, P), :], row)

    if waitfix:
        fix_multiwait(nc)
    return nc


# ---------------------------------------------------------------------------
_NC = None
LAST_RESULTS = None  # BassKernelResults of the most recent kernel() call

F8NP = ml_dtypes.float8_e4m3


def prepare_in_maps(x, mask, Wq, bq, Wk, bk, Wv, bv, Wo, bo, W1, b1, W2, b2,
                    g1, be1, g2, be2):
    bf = ml_dtypes.bfloat16
    x = np.asarray(x, np.float32)
    Wo32 = np.asarray(Wo, np.float32)
    bo_eff = np.asarray(bo, np.float32) + np.asarray(bv, np.float32) @ Wo32

    def col(b_, n, s):  # [n*128] -> [128, n] column layout, scaled by 2^s
        v = np.asarray(b_, np.float32) * (2.0 ** s)
        return np.ascontiguousarray(v.reshape(n, P).T)

    def row(b_, s=0):
        v = np.asarray(b_, np.float32) * (2.0 ** s)
        return np.ascontiguousarray(v.reshape(1, -1))

    def f8(w, s):
        v = np.asarray(w, np.float32) * (2.0 ** s)
        return np.ascontiguousarray(v.astype(F8NP))

    shared = {
        "wq": f8(Wq, SW),
        "wk": f8(Wk, SW),
        "wv": f8(Wv, SW),
        "wo": f8(Wo32, SWO),
        "w1": np.ascontiguousarray(np.asarray(W1, np.float32).astype(bf)),
        "w2": f8(W2, SW2),
        "bqc": col(bq, NDC, SQKV),
        "bkc": col(bk, NDC, SQKV),
        "b1c": col(b1, NFC, SU),
        "b2r": row(b2, SH),
        "g1r": row(g1, SH),
        "be1r": row(be1, SH),
        "g2r": row(g2),
        "be2r": row(be2),
    }

    in_maps = []
    for c in range(8):
        b_, hf = c // 2, c % 2
        xb = x[b_]  # [2048, 1024]
        loc = xb[hf * S_LOC : (hf + 1) * S_LOC, :]
        rem = xb[(1 - hf) * S_LOC : (2 - hf) * S_LOC, :]
        m = dict(shared)
        # token axis rolled: local tokens first (keys are permutation-inv.)
        m["xt"] = np.ascontiguousarray(
            (np.concatenate([loc, rem], axis=0).T * (2.0 ** SX)).astype(F8NP)
        )
        m["xloc"] = np.ascontiguousarray(
            (loc + bo_eff[None, :]) * (2.0 ** SH)
        )
        in_maps.append(m)
    return in_maps


def kernel(x, mask, Wq, bq, Wk, bk, Wv, bv, Wo, bo, W1, b1, W2, b2, g1, be1,
           g2, be2):
    global _NC
    if _NC is None:
        _NC = build_program()
    nc = _NC

    in_maps = prepare_in_maps(x, mask, Wq, bq, Wk, bk, Wv, bv, Wo, bo, W1, b1,
                              W2, b2, g1, be1, g2, be2)
    res = run_bass_kernel_spmd(nc, in_maps, list(range(8)))
    global LAST_RESULTS
    LAST_RESULTS = res

    out = np.empty((4, S_FULL, D), np.float32)
    for c in range(8):
        b_, hf = c // 2, c % 2
        out[b_, hf * S_LOC : (hf + 1) * S_LOC, :] = res.results[c]["out"]
    return out


# revision 29
# speedup vs baseline: 1.9220x; 1.1574x over previous
"""Trainium2 Bass kernel for a post-LN transformer encoder layer.

Reference computation (fp32, per batch b):
    q,k,v = x@Wq+bq, x@Wk+bk, x@Wv+bv          (D=1024, H=16 heads, dk=64)
    attn  = softmax(q k^T / sqrt(dk)) v         (S=2048, mask is all-ones)
    h     = LN(x + attn@Wo + bo; g1, be1)
    out   = LN(h + relu(h@W1+b1)@W2 + b2; g2, be2)

Sharding: 8 cores, fully independent (no collectives). Core c owns batch
b=c//2, sequence half c%2 (1024 query tokens), and redundantly computes
K/V for its full batch (2048 keys) from a host-provided transposed fp8
copy of x (token axis rolled so local tokens come first; attention is
permutation-invariant over keys).

Precision scheme: fp8 e4m3 everywhere the error doesn't reach the output
(the whole attention path contributes ~2% of the residual magnitude) with
DoubleRow double-pumped matmuls (two stacked K=128 k-tiles per
instruction, 2x PE throughput). FFN1 stays bf16 (its input quantization
error would land directly on the output); FFN2 is fp8 DoubleRow. All
quantization scales are powers of two folded into host-side weight prep,
activation-eviction scale immediates, and the LN epsilon, so no extra
on-device ops are spent on scaling. Softmax exp is split across three
engines: exact Exp on ACT, and a Schraudolph bit-trick exp
(bits = scale*score + offset, f32->int8 saturating convert, bitcast to
fp8e4) on Pool and DVE; exp errors cancel between the ctx numerator and
the ones-column denominator.

Scale ledger (powers of 2; value stored = true value * 2^s):
  xt:5  wq/wk/wv:12  QT/KT/VA:5  exp:0  CT:10  wo:3  xloc/hres/hT:13
  w1:0(bf16)  uT:6  w2:7  FFN2 psum:13  LN eps: +26  output: unscaled
"""

import numpy as np
import ml_dtypes

import concourse.bass as bass
import concourse.mybir as mybir
import concourse.tile as tile
from concourse.bass import ts
from concourse.bass_utils import run_bass_kernel_spmd
from concourse.masks import make_identity

BF16 = mybir.dt.bfloat16
F32 = mybir.dt.float32
F32R = mybir.dt.float32r
F8 = mybir.dt.float8e4
I8 = mybir.dt.int8
AF = mybir.ActivationFunctionType
ALU = mybir.AluOpType
DR = mybir.MatmulPerfMode.DoubleRow

D = 1024
DFF = 4096
H = 16
DK = 64
S_FULL = 2048
S_LOC = 1024
P = 128
NDC = D // P        # 8  feature chunks
NFC = DFF // P      # 32 ffn chunks
NKC = S_FULL // P   # 16 key chunks
NTC = S_LOC // P    # 8  local token chunks
NQT = S_LOC // 512  # 2 query tiles of 512

# ---- scale ledger ---------------------------------------------------------
SX = 5        # xt fp8
SW = 12       # wq/wk/wv fp8
SQKV = 5      # QT/KT/VA fp8
SCT = 10      # CT fp8
SWO = 3       # wo fp8
SH = 13       # xloc/hres/hT
SU = 6        # uT fp8
SW2 = 7       # w2 fp8
FFN1_FP8 = True   # fp8 DoubleRow FFN1 (vs bf16); adds ~6e-3 rel error
SW1 = 11      # w1 fp8 scale (fp8 mode)
SHT = 5       # hT fp8 scale (fp8 mode)
SC_QKV_EVICT = 2.0 ** (SQKV - SX - SW)      # psum -> fp8
SC_EXP = 0.125 * 2.0 ** (-2 * SQKV)         # ACT exp scale on scores psum
SC_HT = 2.0 ** (SHT - SH)                   # hres psum -> hT fp8
if FFN1_FP8:
    SC_RELU = 2.0 ** (SU - SHT - SW1)       # FFN1 psum -> uT fp8
else:
    SC_RELU = 2.0 ** (SU - SH)
EPS_LN = 1e-5 * 2.0 ** (2 * SH)             # rows carry 2^SH
LOG2E = 1.4426950408889634
A8 = 8.0 * LOG2E * 0.125 * 2.0 ** (-2 * SQKV)   # Schraudolph mult
B8 = 8.0 * 7.0 - 0.5                            # Schraudolph offset
ONES_R_VAL = float(2 ** (SCT - SQKV))       # PE-bcast recip row scale

# exp engine per key chunk: A=ACT exact, D=DVE Schraudolph. GPSIMD (Pool)
# cannot read PSUM on TRN2, so it gets only SBUF-side work (LN affine etc).
EXP_PAT = ['A', 'D', 'A', 'D', 'A', 'A', 'D', 'A',
           'D', 'A', 'A', 'D', 'A', 'D', 'A', 'D']


# ---------------------------------------------------------------------------
# Multi-wait splitting: this walrus build rejects instructions carrying more
# than one sync-wait command. Tile occasionally emits several (notably the
# kernel-tail drain). Keep the last wait on the instruction and hoist the
# rest onto NoOps inserted just before it on the same engine queue.
_ctr = [0]


def _split_block(bb):
    out = []
    changed = False
    for inst in bb.instructions:
        si = inst.sync_info
        waits = list(si.on_wait) if si is not None and si.on_wait else []
        if len(waits) > 1:
            changed = True
            for w in waits[:-1]:
                _ctr[0] += 1
                nop = mybir.InstNoOp(name=f"waitfix-{_ctr[0]}", ins=[], outs=[])
                nop.engine = inst.engine
                nop.sync_info = mybir.SyncInfo(on_wait=[w], on_update=[])
                out.append(nop)
            inst.sync_info = mybir.SyncInfo(
                on_wait=[waits[-1]], on_update=list(si.on_update or [])
            )
        out.append(inst)
    if changed:
        bb.instructions = out
    return changed


def fix_multiwait(nc):
    for fn in nc.m.functions:
        for bb in fn.blocks:
            _split_block(bb)


# ---------------------------------------------------------------------------
def build_program(reps=1, waitfix=True, phases=('p1', 'attn', 'p3', 'ffn')):
    nc = bass.Bass()

    xt_d = nc.dram_tensor("xt", [D, S_FULL], F8, kind="ExternalInput")
    xloc_d = nc.dram_tensor("xloc", [S_LOC, D], F32, kind="ExternalInput")
    wq_d = nc.dram_tensor("wq", [D, D], F8, kind="ExternalInput")
    wk_d = nc.dram_tensor("wk", [D, D], F8, kind="ExternalInput")
    wv_d = nc.dram_tensor("wv", [D, D], F8, kind="ExternalInput")
    wo_d = nc.dram_tensor("wo", [D, D], F8, kind="ExternalInput")
    w1_d = nc.dram_tensor("w1", [D, DFF], F8 if FFN1_FP8 else BF16,
                          kind="ExternalInput")
    w2_d = nc.dram_tensor("w2", [DFF, D], F8, kind="ExternalInput")
    bqc_d = nc.dram_tensor("bqc", [P, NDC], F32, kind="ExternalInput")
    bkc_d = nc.dram_tensor("bkc", [P, NDC], F32, kind="ExternalInput")
    b1c_d = nc.dram_tensor("b1c", [P, NFC], F32, kind="ExternalInput")
    b2r_d = nc.dram_tensor("b2r", [1, D], F32, kind="ExternalInput")
    g1r_d = nc.dram_tensor("g1r", [1, D], F32, kind="ExternalInput")
    be1r_d = nc.dram_tensor("be1r", [1, D], F32, kind="ExternalInput")
    g2r_d = nc.dram_tensor("g2r", [1, D], F32, kind="ExternalInput")
    be2r_d = nc.dram_tensor("be2r", [1, D], F32, kind="ExternalInput")
    out_d = nc.dram_tensor("out", [S_LOC, D], F32, kind="ExternalOutput")

    xt_r = xt_d.rearrange("(dc p) t -> p dc t", p=P)
    wq_r = wq_d.rearrange("(dc p) o -> p dc o", p=P)
    wk_r = wk_d.rearrange("(dc p) o -> p dc o", p=P)
    wv_r = wv_d.rearrange("(dc p) o -> p dc o", p=P)
    wo_r = wo_d.rearrange("(dc p) o -> p dc o", p=P)
    w1_r = w1_d.rearrange("(dc p) f -> p dc f", p=P)
    w2_r = w2_d.rearrange("(fc p) o -> p fc o", p=P)

    def bcast_row(row_d):
        # [1, D] dram row -> partition-broadcast AP for DMA into [P, D]
        a = row_d[0:1, :]
        return bass.AP(tensor=a.tensor, offset=a.offset, ap=[[0, P], [1, D]])

    with tile.TileContext(nc) as tc:
        with (
            tc.tile_pool(name="top", bufs=1) as top,
            tc.tile_pool(name="lnp", bufs=4) as lnp,
        ):
            # ---- whole-kernel constants / persistents -------------------
            ident = top.tile([P, P], F32)
            make_identity(nc, ident)
            eps_t = top.tile([P, 1], F32)
            nc.vector.memset(eps_t, EPS_LN)
            ones32 = top.tile([1, DK], F32)
            nc.vector.memset(ones32, ONES_R_VAL)
            ones_r = top.tile([1, DK], F32R)
            with nc.allow_low_precision(reason="f32r round for PE broadcast"):
                nc.vector.tensor_copy(ones_r[:], ones32[:])
            bqc = top.tile([P, NDC], F32)
            nc.sync.dma_start(bqc[:], bqc_d[:])
            bkc = top.tile([P, NDC], F32)
            nc.sync.dma_start(bkc[:], bkc_d[:])
            # the remaining constant rows (2.5MB) are DMA'd after p1's
            # critical wq/xt transfers so they don't delay the first matmul
            b1c = top.tile([P, NFC], F32)
            b2b = top.tile([P, D], F32)
            g1b = top.tile([P, D], F32)
            be1b = top.tile([P, D], F32)
            g2b = top.tile([P, D], F32)
            be2b = top.tile([P, D], F32)

            def persist_dmas():
                nc.sync.dma_start(b1c[:], b1c_d[:])
                nc.sync.dma_start(b2b[:], bcast_row(b2r_d))
                nc.sync.dma_start(g1b[:], bcast_row(g1r_d))
                nc.sync.dma_start(be1b[:], bcast_row(be1r_d))
                nc.sync.dma_start(g2b[:], bcast_row(g2r_d))
                nc.sync.dma_start(be2b[:], bcast_row(be2r_d))

            hres = top.tile([P, NTC, D], F32)   # 2^SH * (x+attn), then LN1'd
            # hT split into qt halves so FFN1's first half can start
            # while LN1/transpose still run on the later token chunks
            HT_DT = F8 if FFN1_FP8 else BF16
            hTa = top.tile([P, NDC, 512], HT_DT)
            hTb = top.tile([P, NDC, 512], HT_DT)

            def layernorm_row(row, g_b, be_b, stand=None, mul=None, add=None):
                # stats+recip on DVE; sqrt on ACT; standardize+affine on
                # the given engines (default Pool, the SBUF-only engine)
                stand = stand or nc.gpsimd
                mul = mul or nc.gpsimd
                add = add or nc.gpsimd
                st = lnp.tile([P, 2, 6], F32, tag="st")
                nc.vector.bn_stats(st[:, 0, :], row[:, 0:512])
                nc.vector.bn_stats(st[:, 1, :], row[:, 512:1024])
                mv = lnp.tile([P, 2], F32, tag="mv")
                nc.vector.bn_aggr(mv[:], st[:])
                nc.scalar.activation(mv[:, 1:2], mv[:, 1:2], AF.Sqrt,
                                     bias=eps_t[:])
                nc.vector.reciprocal(mv[:, 1:2], mv[:, 1:2])
                negmur = lnp.tile([P, 1], F32, tag="negmur")
                nc.vector.tensor_scalar(
                    out=negmur[:], in0=mv[:, 0:1],
                    scalar1=mv[:, 1:2], scalar2=-1.0,
                    op0=ALU.mult, op1=ALU.mult,
                )
                # row = row * (1/sigma) + (-mu/sigma)
                if stand is nc.scalar:
                    nc.scalar.activation(row, row, AF.Identity,
                                         bias=negmur[:], scale=mv[:, 1:2])
                else:
                    stand.tensor_scalar(
                        out=row, in0=row,
                        scalar1=mv[:, 1:2], scalar2=negmur[:],
                        op0=ALU.mult, op1=ALU.add,
                    )
                mul.tensor_mul(row, row, g_b[:])
                add.tensor_add(row, row, be_b[:])

            for _rep in range(reps):
                with tc.tile_pool(name="poolA", bufs=1) as poolA:
                    QT = poolA.tile([P, NDC, S_LOC], F8)
                    KT = poolA.tile([P, NDC, S_FULL], F8)
                    VA = poolA.tile([P, NKC, H, DK + 1], F8)
                    CT = poolA.tile([P, NDC, S_LOC], F8)

                    # ---- phase 1: projections -------------------------------
                    if 'p1' in phases:
                      with (
                        tc.tile_pool(name="p1w", bufs=1) as p1w,
                        tc.tile_pool(name="p1x", bufs=1) as p1x,
                        tc.tile_pool(name="p1ps", bufs=8, space="PSUM") as p1ps,
                      ):
                        # one fp8 copy of x^T in SBUF, quartered by tokens
                        xt_q = [p1x.tile([P, NDC, 512], F8, name=f"xtq{i}")
                                for i in range(4)]
                        wq_sb = p1w.tile([P, NDC, D], F8, name="wq_sb")
                        nc.sync.dma_start(xt_q[0][:], xt_r[:, :, 0:512])
                        nc.sync.dma_start(wq_sb[:], wq_r)
                        for i in range(1, 4):
                            nc.sync.dma_start(xt_q[i][:],
                                              xt_r[:, :, ts(i, 512)])
                        wk_sb = p1w.tile([P, NDC, D], F8, name="wk_sb")
                        nc.sync.dma_start(wk_sb[:], wk_r)
                        wv_sb = p1w.tile([P, NDC, D], F8, name="wv_sb")
                        nc.sync.dma_start(wv_sb[:], wv_r)
                        if _rep == 0:
                            persist_dmas()

                        # QT (local tokens = first 1024 cols) and KT (all)
                        for w_sb, bias_c, dst, ntiles, eng in (
                            (wq_sb, bqc, QT, NQT, 'D'),
                            (wk_sb, bkc, KT, NKC // 4, 'A'),
                        ):
                            for kt in range(ntiles):
                                pss = [
                                    p1ps.tile([P, 512], F32, tag="pqk",
                                              name="pqk")
                                    for _ in range(NDC)
                                ]
                                for dcp in range(NDC // 2):
                                    for dc_out in range(NDC):
                                        nc.tensor.matmul(
                                            pss[dc_out][:],
                                            w_sb[:, 2 * dcp : 2 * dcp + 2,
                                                 ts(dc_out, P)],
                                            xt_q[kt][:, 2 * dcp : 2 * dcp + 2, :],
                                            start=(dcp == 0),
                                            stop=(dcp == NDC // 2 - 1),
                                            perf_mode=DR,
                                        )
                                for dc_out in range(NDC):
                                    if eng == 'A':
                                        nc.scalar.activation(
                                            dst[:, dc_out, ts(kt, 512)],
                                            pss[dc_out][:],
                                            AF.Identity,
                                            bias=bias_c[:, dc_out : dc_out + 1],
                                            scale=SC_QKV_EVICT,
                                        )
                                    else:
                                        nc.vector.tensor_scalar(
                                            out=dst[:, dc_out, ts(kt, 512)],
                                            in0=pss[dc_out][:],
                                            scalar1=SC_QKV_EVICT,
                                            scalar2=bias_c[:, dc_out : dc_out + 1],
                                            op0=ALU.mult,
                                            op1=ALU.add,
                                        )

                        # V (natural layout) + ones column
                        for kc in range(NKC):
                            xq = xt_q[kc // 4]
                            for dt_ in range(2):
                                ps = p1ps.tile([P, 512], F32, tag="pqk",
                                               name="pv")
                                for dcp in range(NDC // 2):
                                    nc.tensor.matmul(
                                        ps[:],
                                        xq[:, 2 * dcp : 2 * dcp + 2,
                                           ts(kc % 4, P)],
                                        wv_sb[:, 2 * dcp : 2 * dcp + 2,
                                              ts(dt_, 512)],
                                        start=(dcp == 0),
                                        stop=(dcp == NDC // 2 - 1),
                                        perf_mode=DR,
                                    )
                                nc.scalar.activation(
                                    VA[:, kc, dt_ * 8 : (dt_ + 1) * 8, 0:DK],
                                    ps[:].rearrange("p (h d) -> p h d", h=8),
                                    AF.Copy,
                                    scale=SC_QKV_EVICT,
                                )
                        nc.gpsimd.memset(VA[:, :, :, DK : DK + 1], 1.0)

                    # prefetch wo under the attention phase
                    wop = poolA
                    wo_sb = wop.tile([P, NDC, D], F8)
                    if 'p3' in phases:
                        nc.sync.dma_start(wo_sb[:], wo_r)

                    # ---- phase 2: attention ---------------------------------
                    if 'attn' in phases:
                      with (
                        tc.tile_pool(name="expp", bufs=2) as expp,
                        tc.tile_pool(name="recp", bufs=2) as recp,
                        tc.tile_pool(name="psS", bufs=3, space="PSUM") as psS,
                        tc.tile_pool(name="psC", bufs=1, space="PSUM") as psC,
                      ):
                        cur_pc = {}

                        def ctx_ops(hc, qt, et):
                            pcA = psC.tile([P, 512], F32, tag="ps_cA",
                                           name="ps_cA", bufs=1)
                            pcB = psC.tile([P, 512], F32, tag="ps_cB",
                                           name="ps_cB", bufs=1)
                            cur_pc[(hc, qt)] = (pcA, pcB)
                            ops = []
                            for j, pc in enumerate((pcA, pcB)):
                                h = 2 * hc + j
                                for kp in range(NKC // 2):
                                    ops.append((pc, h, kp, j, et))
                            return ops

                        def emit_ctx_mm(op):
                            pc, h, kp, j, et = op
                            nc.tensor.matmul(
                                pc[0 : DK + 1, :],
                                VA[:, 2 * kp : 2 * kp + 2, h, :],
                                et[:, 2 * kp : 2 * kp + 2, j, :],
                                start=(kp == 0),
                                stop=(kp == NKC // 2 - 1),
                                perf_mode=DR,
                            )

                        def emit_tail(hc, qt):
                            pcA, pcB = cur_pc.pop((hc, qt))
                            for j, pc in enumerate((pcA, pcB)):
                                rec = recp.tile([1, 512], F32R, tag="rec",
                                                name="rec")
                                with nc.allow_low_precision(reason="f32r"):
                                    nc.vector.reciprocal(
                                        rec[:], pc[DK : DK + 1, :]
                                    )
                                # K=1 matmul broadcasts the recip row over dk
                                # partitions (TensorTensor can't read two
                                # PSUMs, SBUF APs can't partition-broadcast)
                                ps_b = psS.tile([DK, 2, 512], F32, tag="ps_s",
                                                name="ps_b")
                                nc.tensor.matmul(
                                    ps_b[:, 0, :], ones_r[:], rec[:],
                                    start=True, stop=True,
                                )
                                recb = recp.tile([DK, 512], F32, tag="recb",
                                                 name="recb")
                                nc.scalar.activation(recb[:], ps_b[:, 0, :],
                                                     AF.Copy)
                                nc.vector.tensor_mul(
                                    CT[DK * j : DK * j + DK, hc, ts(qt, 512)],
                                    pc[0:DK, :],
                                    recb[:],
                                )

                        units = [(hc, qt) for qt in range(NQT)
                                 for hc in range(H // 2)]
                        prev = None
                        for u in units:
                            hc, qt = u
                            et = expp.tile([P, NKC, 2, 512], F8, tag="exp",
                                           name="exp")
                            pops = ctx_ops(*prev) if prev else []
                            pidx = 0
                            for kc in range(NKC):
                                ps_s = psS.tile([P, 2, 512], F32, tag="ps_s",
                                                name="ps_s")
                                for j in range(2):
                                    p0 = DK * j
                                    nc.tensor.matmul(
                                        ps_s[:, j, :],
                                        KT[p0 : p0 + DK, hc, ts(kc, P)],
                                        QT[p0 : p0 + DK, hc, ts(qt, 512)],
                                        start=True,
                                        stop=True,
                                    )
                                eng = EXP_PAT[kc]
                                dst = et[:, kc, :, :]
                                if eng == 'A':
                                    nc.scalar.activation(
                                        dst, ps_s[:], AF.Exp, scale=SC_EXP
                                    )
                                else:
                                    nc.vector.tensor_scalar(
                                        out=dst.bitcast(I8),
                                        in0=ps_s[:],
                                        scalar1=A8,
                                        scalar2=B8,
                                        op0=ALU.mult,
                                        op1=ALU.add,
                                    )
                                if pidx < len(pops):
                                    emit_ctx_mm(pops[pidx])
                                    pidx += 1
                            while pidx < len(pops):
                                emit_ctx_mm(pops[pidx])
                                pidx += 1
                            if prev is not None:
                                emit_tail(*prev[:2])
                            prev = (hc, qt, et)
                        for op in ctx_ops(*prev):
                            emit_ctx_mm(op)
                        emit_tail(*prev[:2])

                    # ---- phase 3: out-proj + residual + LN1 + hT ------------
                    # rows 0-3: full (LN1 + transpose). rows 4-7: out-proj +
                    # residual only; their LN1/transpose is woven into FFN1
                    # where DVE/Pool are otherwise idle.
                    def finish_row(tc_, psT, stand=None, mul=None, add=None):
                        row = hres[:, tc_, :]
                        layernorm_row(row, g1b, be1b, stand, mul, add)
                        hT_half = hTa if tc_ < 4 else hTb
                        tcol = tc_ % 4
                        for dcq in range(2):
                            ps_t = psT.tile([P, 4, P], F32, tag="ps_t",
                                            name="ps_t")
                            for di in range(4):
                                nc.tensor.transpose(
                                    ps_t[:, di, :],
                                    row[:, ts(4 * dcq + di, P)],
                                    ident[:],
                                )
                            if dcq == 0:
                                if FFN1_FP8:
                                    nc.vector.tensor_scalar(
                                        out=hT_half[:, 0:4, ts(tcol, P)],
                                        in0=ps_t[:], scalar1=SC_HT,
                                        scalar2=None, op0=ALU.mult,
                                    )
                                else:
                                    nc.vector.tensor_copy(
                                        hT_half[:, 0:4, ts(tcol, P)], ps_t[:]
                                    )
                            else:
                                nc.scalar.activation(
                                    hT_half[:, 4:8, ts(tcol, P)], ps_t[:],
                                    AF.Copy,
                                    scale=SC_HT if FFN1_FP8 else 1.0,
                                )
                        # pre-add b2 to the residual now that hT holds the
                        # true h'; keeps it off the FFN2 tail path
                        nc.gpsimd.tensor_add(row, row, b2b[:])

                    if 'p3' in phases:
                      with (
                        tc.tile_pool(name="xresp", bufs=3) as xresp,
                        tc.tile_pool(name="psO", bufs=4, space="PSUM") as psO,
                        tc.tile_pool(name="psT", bufs=2, space="PSUM") as psT,
                      ):
                        for tc_ in range(NTC):
                            for dt_ in range(2):
                                ps = psO.tile([P, 512], F32)
                                for dcp in range(NDC // 2):
                                    nc.tensor.matmul(
                                        ps[:],
                                        CT[:, 2 * dcp : 2 * dcp + 2,
                                           ts(tc_, P)],
                                        wo_sb[:, 2 * dcp : 2 * dcp + 2,
                                              ts(dt_, 512)],
                                        start=(dcp == 0),
                                        stop=(dcp == NDC // 2 - 1),
                                        perf_mode=DR,
                                    )
                                xres = xresp.tile([P, 512], F32, tag="xres")
                                nc.sync.dma_start(
                                    xres[:], xloc_d[ts(tc_, P), ts(dt_, 512)]
                                )
                                nc.vector.tensor_add(
                                    hres[:, tc_, ts(dt_, 512)], ps[:], xres[:]
                                )
                            if tc_ < 4:
                                finish_row(tc_, psT)

                # ---- phase 4: FFN (poolA freed) -----------------------------
                if 'ffn' in phases:
                  with (
                    tc.tile_pool(name="uTp", bufs=1) as uTp,
                    tc.tile_pool(name="w1p", bufs=3) as w1p,
                    tc.tile_pool(name="w2p", bufs=1) as w2p,
                    tc.tile_pool(name="psF", bufs=4, space="PSUM") as psF,
                    tc.tile_pool(name="psT2", bufs=2, space="PSUM") as psT2,
                  ):
                    uT = uTp.tile([P, NFC, S_LOC], F8)
                    # w2 is 4MB fp8: fetch both halves under FFN1's compute
                    w2_sb = w2p.tile([P, NFC, D], F8, name="w2sb")
                    nc.sync.dma_start(w2_sb[:], w2_r)

                    def ffn1_mm(fc, qt, w1_sb):
                        hT_half = hTa if qt == 0 else hTb
                        ps = psF.tile([P, 512], F32, tag="psf", name="psf")
                        if FFN1_FP8:
                            for dcp in range(NDC // 2):
                                nc.tensor.matmul(
                                    ps[:],
                                    w1_sb[:, 2 * dcp : 2 * dcp + 2, :],
                                    hT_half[:, 2 * dcp : 2 * dcp + 2, :],
                                    start=(dcp == 0),
                                    stop=(dcp == NDC // 2 - 1),
                                    perf_mode=DR,
                                )
                        else:
                            for dc in range(NDC):
                                nc.tensor.matmul(
                                    ps[:],
                                    w1_sb[:, dc, :],
                                    hT_half[:, dc, :],
                                    start=(dc == 0),
                                    stop=(dc == NDC - 1),
                                )
                        nc.scalar.activation(
                            uT[:, fc, ts(qt, 512)],
                            ps[:],
                            AF.Relu,
                            bias=b1c[:, fc : fc + 1],
                            scale=SC_RELU,
                        )

                    # pass A: fc 0-7 on the ready hTa half, weaving in the
                    # deferred LN1/transpose of rows 4-7
                    for fc in range(8):
                        w1_sb = w1p.tile([P, NDC, P], F8 if FFN1_FP8 else BF16, tag="w1")
                        nc.sync.dma_start(w1_sb[:], w1_r[:, :, ts(fc, P)])
                        ffn1_mm(fc, 0, w1_sb)
                        if fc % 2 == 1 and 'p3' in phases:
                            finish_row(4 + fc // 2, psT2,
                                       stand=nc.scalar, mul=nc.vector,
                                       add=nc.gpsimd)
                    # pass B: fc 8-31, both token halves
                    for fc in range(8, NFC):
                        w1_sb = w1p.tile([P, NDC, P], F8 if FFN1_FP8 else BF16, tag="w1")
                        nc.sync.dma_start(w1_sb[:], w1_r[:, :, ts(fc, P)])
                        ffn1_mm(fc, 0, w1_sb)
                        ffn1_mm(fc, 1, w1_sb)
                    # pass C: fc 0-7 on hTb (w1 restreamed, +2MB DMA)
                    for fc in range(8):
                        w1_sb = w1p.tile([P, NDC, P], F8 if FFN1_FP8 else BF16, tag="w1")
                        nc.sync.dma_start(w1_sb[:], w1_r[:, :, ts(fc, P)])
                        ffn1_mm(fc, 1, w1_sb)

                    y = uTp.tile([P, NTC, D], F32)
                    with (
                        tc.tile_pool(name="psY", bufs=2, space="PSUM") as psY,
                    ):
                        # token-chunk outer so each row's LN2 + store
                        # pipelines under the next rows' matmuls
                        for tc_ in range(NTC):
                            for dt_ in range(2):
                                ps = psY.tile([P, 512], F32)
                                for fcp in range(NFC // 2):
                                    nc.tensor.matmul(
                                        ps[:],
                                        uT[:, 2 * fcp : 2 * fcp + 2,
                                           ts(tc_, P)],
                                        w2_sb[:, 2 * fcp : 2 * fcp + 2,
                                              ts(dt_, 512)],
                                        start=(fcp == 0),
                                        stop=(fcp == NFC // 2 - 1),
                                        perf_mode=DR,
                                    )
                                nc.vector.tensor_add(
                                    y[:, tc_, ts(dt_, 512)],
                                    ps[:],
                                    hres[:, tc_, ts(dt_, 512)],
                                )
                            row = y[:, tc_, :]
                            layernorm_row(row, g2b, be2b)
                            nc.sync.dma_start(out_d[ts(tc_, P), :], row)

    if waitfix:
        fix_multiwait(nc)
    return nc


# ---------------------------------------------------------------------------
_NC = None
LAST_RESULTS = None  # BassKernelResults of the most recent kernel() call

F8NP = ml_dtypes.float8_e4m3


def prepare_in_maps(x, mask, Wq, bq, Wk, bk, Wv, bv, Wo, bo, W1, b1, W2, b2,
                    g1, be1, g2, be2):
    bf = ml_dtypes.bfloat16
    x = np.asarray(x, np.float32)
    Wo32 = np.asarray(Wo, np.float32)
    bo_eff = np.asarray(bo, np.float32) + np.asarray(bv, np.float32) @ Wo32

    def col(b_, n, s):  # [n*128] -> [128, n] column layout, scaled by 2^s
        v = np.asarray(b_, np.float32) * (2.0 ** s)
        return np.ascontiguousarray(v.reshape(n, P).T)

    def row(b_, s=0):
        v = np.asarray(b_, np.float32) * (2.0 ** s)
        return np.ascontiguousarray(v.reshape(1, -1))

    def f8(w, s):
        v = np.asarray(w, np.float32) * (2.0 ** s)
        return np.ascontiguousarray(v.astype(F8NP))

    shared = {
        "wq": f8(Wq, SW),
        "wk": f8(Wk, SW),
        "wv": f8(Wv, SW),
        "wo": f8(Wo32, SWO),
        "w1": f8(W1, SW1) if FFN1_FP8 else
            np.ascontiguousarray(np.asarray(W1, np.float32).astype(bf)),
        "w2": f8(W2, SW2),
        "bqc": col(bq, NDC, SQKV),
        "bkc": col(bk, NDC, SQKV),
        "b1c": col(b1, NFC, SU),
        "b2r": row(b2, SH),
        "g1r": row(g1, SH),
        "be1r": row(be1, SH),
        "g2r": row(g2),
        "be2r": row(be2),
    }

    in_maps = []
    for c in range(8):
        b_, hf = c // 2, c % 2
        xb = x[b_]  # [2048, 1024]
        loc = xb[hf * S_LOC : (hf + 1) * S_LOC, :]
        rem = xb[(1 - hf) * S_LOC : (2 - hf) * S_LOC, :]
        m = dict(shared)
        # token axis rolled: local tokens first (keys are permutation-inv.)
        m["xt"] = np.ascontiguousarray(
            (np.concatenate([loc, rem], axis=0).T * (2.0 ** SX)).astype(F8NP)
        )
        m["xloc"] = np.ascontiguousarray(
            (loc + bo_eff[None, :]) * (2.0 ** SH)
        )
        in_maps.append(m)
    return in_maps


def kernel(x, mask, Wq, bq, Wk, bk, Wv, bv, Wo, bo, W1, b1, W2, b2, g1, be1,
           g2, be2):
    global _NC
    if _NC is None:
        _NC = build_program()
    nc = _NC

    in_maps = prepare_in_maps(x, mask, Wq, bq, Wk, bk, Wv, bv, Wo, bo, W1, b1,
                              W2, b2, g1, be1, g2, be2)
    res = run_bass_kernel_spmd(nc, in_maps, list(range(8)))
    global LAST_RESULTS
    LAST_RESULTS = res

    out = np.empty((4, S_FULL, D), np.float32)
    for c in range(8):
        b_, hf = c // 2, c % 2
        out[b_, hf * S_LOC : (hf + 1) * S_LOC, :] = res.results[c]["out"]
    return out
